# revision 8
# baseline (speedup 1.0000x reference)
"""Trainium2 Bass kernel for AttentionConvFull (local 5x5 window attention
with per-channel softmax, grouped 1x1 conv projections).

Sharding: 8 cores = batch(4) x H-halves(2). Each core gets a 32-row halo'd,
zero-padded slice of x, pre-transposed on host to channel-major [256, 32*60].
No collectives needed.

V2 dataflow per core (2 channel-chunks of 128 partitions each):
  PE    : block-diag 128x128 bf16 matmuls for q/k/v projections; per window
          offset j, identity-matmul PSUM accumulation of den += e_j and
          num += (e_j * v_j).
  DVE   : fused scalar_tensor_tensor t_j = (k_j + rel_j) * q (one pass,
          replaces the separate rel-add); w_j = e_j * v_j tensor_tensor;
          PSUM->SBUF projection copies.
  ACT   : exp over j-PAIRS (batched to amortize per-instr overhead); q
          projection copy with fused q_emb bias.
  GPSIMD: takes a subset of the w_j multiplies to relieve DVE.
  DMA   : 1-elem-shifted copies of k/v maps so odd window columns keep 4B
          alignment (DVE 2x mode); bf16 output (host upcasts to f32).
"""

import numpy as np
import ml_dtypes

import concourse.bass as bass
import concourse.tile as tile
from concourse import bacc, mybir
from concourse.bass_utils import run_bass_kernel_spmd

F32 = mybir.dt.float32
BF16 = mybir.dt.bfloat16

K = 5
G = 8
B, H, W, C = 4, 56, 56, 256
Cg = C // G            # 32
P = K // 2             # 2
HS = H // 2            # 28 output rows per shard
MR = HS + 2 * P        # 32 map rows
MC = W + 2 * P         # 60 map cols
SP = MR * MC           # 1920 map spatial
OP = HS * W            # 1568 output spatial per shard
NCH = 2                # channel chunks of 128 partitions
NCORES = 8
HALF = OP // 2         # 784: PSUM accumulate tile half-size

# which j-iterations (0..24, dj-major order position) run their w-multiply
# on GPSIMD instead of DVE
GP_POS = {1, 4, 8, 12, 16, 20, 23}
# which j-iterations run their kr-add (k + rel) on GPSIMD instead of DVE
GP_KR = {2, 9, 14, 19, 24}


def _dedup_ldweights(nc):
    """Remove redundant PE weight reloads: consecutive InstLdweights that
    load the same stationary operand with no sync info."""
    removed = 0
    for blk in nc.main_func.blocks:
        last_sig = None
        keep = []
        for inst in blk.instructions:
            if isinstance(inst, mybir.InstLdweights):
                sig = " ".join(a.concise() for a in inst.ins)
                si = inst.sync_info
                clean = si is None or (
                    len(si.on_wait) == 0 and len(si.on_update) == 0
                )
                if sig == last_sig and clean:
                    removed += 1
                    continue
                last_sig = sig
            elif isinstance(inst, mybir.InstMatmult):
                if len(inst.ins) > 1:
                    wsig = inst.ins[1].concise()
                    if wsig != last_sig:
                        last_sig = wsig
            keep.append(inst)
        blk.instructions[:] = keep
    return removed


def build_nc():
    nc = bacc.Bacc(
        "TRN2", target_bir_lowering=False, debug=False, num_devices=NCORES
    )

    xt_d = nc.dram_tensor("xt", [NCH, 128, SP], BF16, kind="ExternalInput").ap()
    wq_d = nc.dram_tensor("wqb", [NCH, 128, 128], BF16, kind="ExternalInput").ap()
    wk_d = nc.dram_tensor("wkb", [NCH, 128, 128], BF16, kind="ExternalInput").ap()
    wv_d = nc.dram_tensor("wvb", [NCH, 128, 128], BF16, kind="ExternalInput").ap()
    rel_d = nc.dram_tensor("relb", [NCH, 128, K * K], F32, kind="ExternalInput").ap()
    qe_d = nc.dram_tensor("qeb", [NCH, 128, 1], F32, kind="ExternalInput").ap()
    id_d = nc.dram_tensor("idn", [128, 128], BF16, kind="ExternalInput").ap()
    out_d = nc.dram_tensor("out", [NCH, 128, OP], BF16, kind="ExternalOutput").ap()

    # dj-major j order: odd-shifted maps (needed from dj=1) arrive via DMA
    # while dj=0 iterations run
    JLIST = [(di, dj) for dj in range(K) for di in range(K)]

    with tile.TileContext(nc) as tc:
        with (
            tc.tile_pool(name="consts", bufs=1) as consts,
            tc.tile_pool(name="weights", bufs=2) as wpool,
            tc.tile_pool(name="xin", bufs=2) as xpool,
            tc.tile_pool(name="maps", bufs=2) as mpool,
            tc.tile_pool(name="jwork", bufs=3) as jpool,
            tc.tile_pool(name="epi", bufs=2) as epool,
            tc.tile_pool(name="acc", bufs=4, space=bass.MemorySpace.PSUM) as psum,
        ):
            ident = consts.tile([128, 128], BF16, tag="ident")
            nc.sync.dma_start(ident[:], id_d)

            kmaps, komaps, vmaps, vomaps, qflats, rels = [], [], [], [], [], []

            for c in range(NCH):
                x_sb = xpool.tile([128, SP], BF16, tag="x")
                nc.sync.dma_start(x_sb[:], xt_d[c])

                wts = {}
                for nm, d in (("wq", wq_d), ("wk", wk_d), ("wv", wv_d)):
                    t = wpool.tile([128, 128], BF16, tag=nm, name=f"{nm}{c}")
                    nc.sync.dma_start(t[:], d[c])
                    wts[nm] = t
                rel_sb = wpool.tile([128, K * K], F32, tag="rel", name=f"rel{c}")
                nc.sync.dma_start(rel_sb[:], rel_d[c])
                qe_sb = wpool.tile([128, 1], F32, tag="qe", name=f"qe{c}")
                nc.sync.dma_start(qe_sb[:], qe_d[c])
                rels.append(rel_sb)

                k_bf = mpool.tile([128, SP], BF16, tag="k", name=f"k{c}")
                v_bf = mpool.tile([128, SP], BF16, tag="v", name=f"v{c}")
                qf = mpool.tile([128, OP], BF16, tag="qf", name=f"qf{c}")

                # projections: weight-major to minimize ldweights; 2 psum
                # tiles of 960 cols (16 map rows) each
                NS = 2
                SL = SP // NS  # 960
                for nm in ("wk", "wv", "wq"):
                    for s in range(NS):
                        lo = s * SL
                        rhs = x_sb[:, lo : lo + SL]
                        ps = psum.tile(
                            [128, SL], F32, tag="acc", name=f"pp{c}{s}{nm}"
                        )
                        for mlo, mn in ((0, 512), (512, SL - 512)):
                            nc.tensor.matmul(
                                ps[:, mlo : mlo + mn],
                                wts[nm][:],
                                rhs[:, mlo : mlo + mn],
                                start=True,
                                stop=True,
                            )
                        if nm == "wq":
                            # write the interior (h in [2,30), w in [2,58))
                            # of this 16-row band directly into flat q,
                            # fusing the q_emb per-partition bias (ACT)
                            r0 = max(P, 16 * s)
                            r1 = min(MR - P, 16 * (s + 1))
                            src = ps[:].rearrange("p (h w) -> p h w", h=16)[
                                :, r0 - 16 * s : r1 - 16 * s, P : P + W
                            ]
                            dst = qf[:].rearrange("p (h w) -> p h w", h=HS)[
                                :, r0 - P : r1 - P, :
                            ]
                            nc.scalar.activation(
                                dst,
                                src,
                                mybir.ActivationFunctionType.Identity,
                                bias=qe_sb[:],
                            )
                        elif nm == "wk":
                            nc.scalar.copy(k_bf[:, lo : lo + SL], ps[:])
                        else:
                            nc.scalar.copy(v_bf[:, lo : lo + SL], ps[:])

                # 1-elem-shifted copies (contiguous SBUF->SBUF DMA):
                # x_od[i] = x[i+1], so odd-dj window reads stay 4B-aligned
                k_od = mpool.tile([128, SP], BF16, tag="ko", name=f"ko{c}")
                v_od = mpool.tile([128, SP], BF16, tag="vo", name=f"vo{c}")
                nc.sync.dma_start(k_od[:, : SP - 1], k_bf[:, 1:])
                nc.sync.dma_start(v_od[:, : SP - 1], v_bf[:, 1:])

                kmaps.append(k_bf); komaps.append(k_od)
                vmaps.append(v_bf); vomaps.append(v_od)
                qflats.append(qf)

            # ---- j-loop per chunk ----
            for c in range(NCH):
                rel_sb, qf = rels[c], qflats[c]

                den = [
                    psum.tile([128, HALF], F32, tag="acc", name=f"den{c}{h}")
                    for h in range(2)
                ]
                num = [
                    psum.tile([128, HALF], F32, tag="acc", name=f"num{c}{h}")
                    for h in range(2)
                ]

                # pairs of consecutive j positions share one exp instruction
                pairs = [JLIST[i : i + 2] for i in range(0, len(JLIST), 2)]
                pos = 0
                for pr in pairs:
                    npr = len(pr)
                    t2 = jpool.tile(
                        [128, npr * OP], BF16, tag="t2", name=f"t{c}{pos}"
                    )
                    e2 = jpool.tile(
                        [128, npr * OP], BF16, tag="e2", name=f"e{c}{pos}"
                    )
                    vviews = []
                    # kr = k_j + rel_j (TS, 4x mode) then t = kr * q (TT, 2x)
                    for i, (di, dj) in enumerate(pr):
                        j = di * K + dj
                        if dj % 2 == 0:
                            ksrc, vsrc, dje = kmaps[c], vmaps[c], dj
                        else:
                            ksrc, vsrc, dje = komaps[c], vomaps[c], dj - 1
                        k3 = ksrc[:].rearrange("p (h w) -> p h w", h=MR)
                        v3 = vsrc[:].rearrange("p (h w) -> p h w", h=MR)
                        kv = k3[:, di : di + HS, dje : dje + W]
                        vviews.append(v3[:, di : di + HS, dje : dje + W])
                        kr_t = jpool.tile(
                            [128, OP], BF16, tag="kr", name=f"kr{c}{pos + i}"
                        )
                        kr3 = kr_t[:].rearrange("p (h w) -> p h w", h=HS)
                        keng = (
                            nc.gpsimd if (pos + i) in GP_KR else nc.vector
                        )
                        keng.tensor_scalar(
                            kr3,
                            kv,
                            rel_sb[:, j : j + 1],
                            None,
                            mybir.AluOpType.add,
                        )
                        nc.vector.tensor_tensor(
                            t2[:, i * OP : (i + 1) * OP],
                            kr_t[:],
                            qf[:],
                            mybir.AluOpType.mult,
                        )

                    # one exp pass for the pair
                    nc.scalar.activation(
                        e2[:], t2[:], mybir.ActivationFunctionType.Exp
                    )

                    for i, (di, dj) in enumerate(pr):
                        eflat = e2[:, i * OP : (i + 1) * OP]
                        e3 = eflat.rearrange("p (h w) -> p h w", h=HS)
                        w_t = jpool.tile(
                            [128, OP], BF16, tag="w", name=f"w{c}{pos + i}"
                        )
                        w3 = w_t[:].rearrange("p (h w) -> p h w", h=HS)
                        eng = (
                            nc.gpsimd if (pos + i) in GP_POS else nc.vector
                        )
                        eng.tensor_tensor(
                            w3, e3, vviews[i], mybir.AluOpType.mult
                        )

                        st = pos + i == 0
                        sp = pos + i == K * K - 1
                        for acc, src_t in ((den, eflat), (num, w_t[:])):
                            for h in range(2):
                                base = h * HALF
                                for lo, n in ((0, 512), (512, HALF - 512)):
                                    nc.tensor.matmul(
                                        acc[h][:, lo : lo + n],
                                        ident[:],
                                        src_t[:, base + lo : base + lo + n],
                                        start=st,
                                        stop=sp,
                                    )
                    pos += npr

                # ---- epilogue ----
                out_sb = epool.tile([128, OP], BF16, tag="osb", name=f"osb{c}")
                for h in range(2):
                    base = h * HALF
                    rden = epool.tile([128, HALF], F32, tag="rden", name=f"rd{c}{h}")
                    nc.vector.reciprocal_approx_fast(rden[:], den[h][:])
                    nc.vector.tensor_tensor(
                        out_sb[:, base : base + HALF],
                        num[h][:],
                        rden[:],
                        mybir.AluOpType.mult,
                    )
                nc.sync.dma_start(out_d[c], out_sb[:])

    nc.compile()
    _dedup_ldweights(nc)
    return nc


def _block_diag_weights(w):
    """w: (G, Cg_out, Cg_in) -> lhsT layout [NCH, 128, 128] where
    lhsT[c, ci, co] = w[g, co%32, ci%32] for matching 32-blocks."""
    out = np.zeros((NCH, 128, 128), np.float32)
    for c in range(NCH):
        for g4 in range(4):
            g = c * 4 + g4
            blk = w[g]  # (Cg_out, Cg_in)
            out[c, g4 * 32 : (g4 + 1) * 32, g4 * 32 : (g4 + 1) * 32] = blk.T
    return out


_NC_CACHE = {}


def _make_in_maps(inputs):
    x = np.asarray(inputs["x"], np.float32)
    wq = np.asarray(inputs["wq"], np.float32)
    wk = np.asarray(inputs["wk"], np.float32)
    wv = np.asarray(inputs["wv"], np.float32)
    rel_emb = np.asarray(inputs["rel_emb"], np.float32)
    q_emb = np.asarray(inputs["q_emb"], np.float32)

    bf = ml_dtypes.bfloat16
    wqb = _block_diag_weights(wq).astype(bf)
    wkb = _block_diag_weights(wk).astype(bf)
    wvb = _block_diag_weights(wv).astype(bf)
    relb = np.ascontiguousarray(
        rel_emb.reshape(G, Cg, K * K).reshape(NCH, 128, K * K)
    )
    qeb = np.ascontiguousarray(q_emb.reshape(NCH, 128, 1))
    idn = np.eye(128, dtype=bf)

    xp = np.pad(x, ((0, 0), (P, P), (P, P), (0, 0)))  # (B, 60, 60, C)

    in_maps = []
    for core in range(NCORES):
        b, half = divmod(core, 2)
        sh = xp[b, HS * half : HS * half + MR]         # (32, 60, C)
        xt = np.ascontiguousarray(sh.reshape(SP, C).T).reshape(NCH, 128, SP)
        in_maps.append(
            {
                "xt": xt.astype(bf),
                "wqb": wqb,
                "wkb": wkb,
                "wvb": wvb,
                "relb": relb,
                "qeb": qeb,
                "idn": idn,
            }
        )
    return in_maps


def kernel(**inputs):
    in_maps = _make_in_maps(inputs)

    if "nc" not in _NC_CACHE:
        _NC_CACHE["nc"] = build_nc()
    nc = _NC_CACHE["nc"]

    res = run_bass_kernel_spmd(nc, in_maps, core_ids=list(range(NCORES)))

    out = np.empty((B, H, W, C), np.float32)
    for core in range(NCORES):
        b, half = divmod(core, 2)
        o = np.asarray(res.results[core]["out"]).astype(np.float32)
        o = o.reshape(C, HS, W)
        out[b, HS * half : HS * half + HS] = o.transpose(1, 2, 0)
    return out


# revision 13
# speedup vs baseline: 2.0428x; 2.0428x over previous
"""Trainium2 Bass kernel for AttentionConvFull (local 5x5 window attention
with per-channel softmax, grouped 1x1 conv projections).

Sharding: 8 cores = batch(4) x H-halves(2). Each core gets a 32-row halo'd,
zero-padded slice of x, pre-transposed on host to channel-major [256, 32*60].
No collectives needed.

V2 dataflow per core (2 channel-chunks of 128 partitions each):
  PE    : block-diag 128x128 bf16 matmuls for q/k/v projections; per window
          offset j, identity-matmul PSUM accumulation of den += e_j and
          num += (e_j * v_j).
  DVE   : fused scalar_tensor_tensor t_j = (k_j + rel_j) * q (one pass,
          replaces the separate rel-add); w_j = e_j * v_j tensor_tensor;
          PSUM->SBUF projection copies.
  ACT   : exp over j-PAIRS (batched to amortize per-instr overhead); q
          projection copy with fused q_emb bias.
  GPSIMD: takes a subset of the w_j multiplies to relieve DVE.
  DMA   : 1-elem-shifted copies of k/v maps so odd window columns keep 4B
          alignment (DVE 2x mode); bf16 output (host upcasts to f32).
"""

import numpy as np
import ml_dtypes

import concourse.bass as bass
import concourse.tile as tile
from concourse import bacc, mybir
from concourse.bass_utils import run_bass_kernel_spmd

F32 = mybir.dt.float32
BF16 = mybir.dt.bfloat16

K = 5
G = 8
B, H, W, C = 4, 56, 56, 256
Cg = C // G            # 32
P = K // 2             # 2
HS = H // 2            # 28 output rows per shard
MR = HS + 2 * P        # 32 map rows
MC = W + 2 * P         # 60 map cols
SP = MR * MC           # 1920 map spatial
OP = HS * W            # 1568 output spatial per shard
NCH = 2                # channel chunks of 128 partitions
NCORES = 8
HALF = OP // 2         # 784: PSUM accumulate tile half-size

# which j-iterations (0..24, dj-major order position) run their w-multiply
# on GPSIMD instead of DVE (GPSIMD TT ~3.3us/pass vs DVE 0.95us; relieves DVE)
GP_POS = {1, 4, 7, 10, 13, 16, 19, 22}
# which j-iterations run their kr-add on ACT (Identity + per-partition bias)
# instead of DVE tensor_scalar; GPSIMD tensor_scalar is pathological (24us)
ACT_KR = {0, 3, 6, 9, 12, 15, 18, 21}
# j's per exp batch (one ACT instruction)
EXPB = 4


def _dedup_ldweights(nc):
    """Remove redundant PE weight reloads: consecutive InstLdweights that
    load the same stationary operand with no sync info."""
    removed = 0
    for blk in nc.main_func.blocks:
        last_sig = None
        keep = []
        for inst in blk.instructions:
            if isinstance(inst, mybir.InstLdweights):
                sig = " ".join(a.concise() for a in inst.ins)
                si = inst.sync_info
                clean = si is None or (
                    len(si.on_wait) == 0 and len(si.on_update) == 0
                )
                if sig == last_sig and clean:
                    removed += 1
                    continue
                last_sig = sig
            elif isinstance(inst, mybir.InstMatmult):
                if len(inst.ins) > 1:
                    wsig = inst.ins[1].concise()
                    if wsig != last_sig:
                        last_sig = wsig
            keep.append(inst)
        blk.instructions[:] = keep
    return removed


def build_nc():
    nc = bacc.Bacc(
        "TRN2", target_bir_lowering=False, debug=False, num_devices=NCORES
    )

    xt_d = nc.dram_tensor("xt", [NCH, 128, SP], BF16, kind="ExternalInput").ap()
    wq_d = nc.dram_tensor("wqb", [NCH, 128, 128], BF16, kind="ExternalInput").ap()
    wk_d = nc.dram_tensor("wkb", [NCH, 128, 128], BF16, kind="ExternalInput").ap()
    wv_d = nc.dram_tensor("wvb", [NCH, 128, 128], BF16, kind="ExternalInput").ap()
    rel_d = nc.dram_tensor("relb", [NCH, 128, K * K], F32, kind="ExternalInput").ap()
    qe_d = nc.dram_tensor("qeb", [NCH, 128, 1], F32, kind="ExternalInput").ap()
    id_d = nc.dram_tensor("idn", [128, 128], BF16, kind="ExternalInput").ap()
    out_d = nc.dram_tensor("out", [NCH, 128, OP], BF16, kind="ExternalOutput").ap()

    # dj-major j order: odd-shifted maps (needed from dj=1) arrive via DMA
    # while dj=0 iterations run
    JLIST = [(di, dj) for dj in range(K) for di in range(K)]

    with tile.TileContext(nc) as tc:
        with (
            tc.tile_pool(name="consts", bufs=1) as consts,
            tc.tile_pool(name="weights", bufs=2) as wpool,
            tc.tile_pool(name="xin", bufs=2) as xpool,
            tc.tile_pool(name="maps", bufs=2) as mpool,
            tc.tile_pool(name="jwork", bufs=4) as jpool,
            tc.tile_pool(name="qwork", bufs=2) as qpool,
            tc.tile_pool(name="epi", bufs=2) as epool,
            tc.tile_pool(name="acc", bufs=4, space=bass.MemorySpace.PSUM) as psum,
        ):
            ident = consts.tile([128, 128], BF16, tag="ident")
            nc.sync.dma_start(ident[:], id_d)

            kmaps, komaps, vmaps, vomaps, qflats, rels = [], [], [], [], [], []

            for c in range(NCH):
                x_sb = xpool.tile([128, SP], BF16, tag="x")
                # split the input DMA so the first projection segment can
                # start as soon as its half lands
                hsp = SP // 2
                nc.sync.dma_start(x_sb[:, :hsp], xt_d[c][:, :hsp])
                nc.sync.dma_start(x_sb[:, hsp:], xt_d[c][:, hsp:])

                wts = {}
                for nm, d in (("wq", wq_d), ("wk", wk_d), ("wv", wv_d)):
                    t = wpool.tile([128, 128], BF16, tag=nm, name=f"{nm}{c}")
                    nc.sync.dma_start(t[:], d[c])
                    wts[nm] = t
                rel_sb = wpool.tile([128, K * K], F32, tag="rel", name=f"rel{c}")
                nc.sync.dma_start(rel_sb[:], rel_d[c])
                qe_sb = wpool.tile([128, 1], F32, tag="qe", name=f"qe{c}")
                nc.sync.dma_start(qe_sb[:], qe_d[c])
                rels.append(rel_sb)

                k_bf = mpool.tile([128, SP], BF16, tag="k", name=f"k{c}")
                v_bf = mpool.tile([128, SP], BF16, tag="v", name=f"v{c}")
                qf = mpool.tile([128, OP], BF16, tag="qf", name=f"qf{c}")

                # projections: weight-major to minimize ldweights; 2 psum
                # tiles of 960 cols (16 map rows) each
                NS = 2
                SL = SP // NS  # 960
                for nm in ("wk", "wv", "wq"):
                    for s in range(NS):
                        lo = s * SL
                        rhs = x_sb[:, lo : lo + SL]
                        ps = psum.tile(
                            [128, SL], F32, tag="acc", name=f"pp{c}{s}{nm}"
                        )
                        for mlo, mn in ((0, 512), (512, SL - 512)):
                            nc.tensor.matmul(
                                ps[:, mlo : mlo + mn],
                                wts[nm][:],
                                rhs[:, mlo : mlo + mn],
                                start=True,
                                stop=True,
                            )
                        if nm == "wq":
                            # write the interior (h in [2,30), w in [2,58))
                            # of this 16-row band directly into flat q,
                            # fusing the q_emb per-partition bias (ACT)
                            r0 = max(P, 16 * s)
                            r1 = min(MR - P, 16 * (s + 1))
                            src = ps[:].rearrange("p (h w) -> p h w", h=16)[
                                :, r0 - 16 * s : r1 - 16 * s, P : P + W
                            ]
                            dst = qf[:].rearrange("p (h w) -> p h w", h=HS)[
                                :, r0 - P : r1 - P, :
                            ]
                            nc.scalar.activation(
                                dst,
                                src,
                                mybir.ActivationFunctionType.Identity,
                                bias=qe_sb[:],
                            )
                        elif nm == "wk":
                            nc.scalar.copy(k_bf[:, lo : lo + SL], ps[:])
                        else:
                            nc.scalar.copy(v_bf[:, lo : lo + SL], ps[:])

                # 1-elem-shifted copies (contiguous SBUF->SBUF DMA):
                # x_od[i] = x[i+1], so odd-dj window reads stay 4B-aligned
                k_od = mpool.tile([128, SP], BF16, tag="ko", name=f"ko{c}")
                v_od = mpool.tile([128, SP], BF16, tag="vo", name=f"vo{c}")
                nc.sync.dma_start(k_od[:, : SP - 1], k_bf[:, 1:])
                nc.sync.dma_start(v_od[:, : SP - 1], v_bf[:, 1:])

                kmaps.append(k_bf); komaps.append(k_od)
                vmaps.append(v_bf); vomaps.append(v_od)
                qflats.append(qf)

            # ---- j-loop per chunk ----
            for c in range(NCH):
                rel_sb, qf = rels[c], qflats[c]

                den = [
                    psum.tile([128, HALF], F32, tag="acc", name=f"den{c}{h}")
                    for h in range(2)
                ]
                num = [
                    psum.tile([128, HALF], F32, tag="acc", name=f"num{c}{h}")
                    for h in range(2)
                ]

                # batches of EXPB j positions share one exp instruction
                batches = [JLIST[i : i + EXPB] for i in range(0, len(JLIST), EXPB)]
                pos = 0
                for pr in batches:
                    npr = len(pr)
                    t2 = qpool.tile(
                        [128, npr * OP], BF16, tag="t2", name=f"t{c}{pos}"
                    )
                    e2 = qpool.tile(
                        [128, npr * OP], BF16, tag="e2", name=f"e{c}{pos}"
                    )
                    vviews = []
                    # kr = k_j + rel_j (DVE TS 4x / ACT bias), t = kr * q (TT 2x)
                    for i, (di, dj) in enumerate(pr):
                        j = di * K + dj
                        if dj % 2 == 0:
                            ksrc, vsrc, dje = kmaps[c], vmaps[c], dj
                        else:
                            ksrc, vsrc, dje = komaps[c], vomaps[c], dj - 1
                        k3 = ksrc[:].rearrange("p (h w) -> p h w", h=MR)
                        v3 = vsrc[:].rearrange("p (h w) -> p h w", h=MR)
                        kv = k3[:, di : di + HS, dje : dje + W]
                        vviews.append(v3[:, di : di + HS, dje : dje + W])
                        kr_t = jpool.tile(
                            [128, OP], BF16, tag="kr", name=f"kr{c}{pos + i}"
                        )
                        kr3 = kr_t[:].rearrange("p (h w) -> p h w", h=HS)
                        if (pos + i) in ACT_KR:
                            nc.scalar.activation(
                                kr3,
                                kv,
                                mybir.ActivationFunctionType.Identity,
                                bias=rel_sb[:, j : j + 1],
                            )
                        else:
                            nc.vector.tensor_scalar(
                                kr3,
                                kv,
                                rel_sb[:, j : j + 1],
                                None,
                                mybir.AluOpType.add,
                            )
                        nc.vector.tensor_tensor(
                            t2[:, i * OP : (i + 1) * OP],
                            kr_t[:],
                            qf[:],
                            mybir.AluOpType.mult,
                        )

                    # one exp pass for the batch
                    nc.scalar.activation(
                        e2[:], t2[:], mybir.ActivationFunctionType.Exp
                    )

                    for i, (di, dj) in enumerate(pr):
                        eflat = e2[:, i * OP : (i + 1) * OP]
                        e3 = eflat.rearrange("p (h w) -> p h w", h=HS)
                        w_t = jpool.tile(
                            [128, OP], BF16, tag="w", name=f"w{c}{pos + i}"
                        )
                        w3 = w_t[:].rearrange("p (h w) -> p h w", h=HS)
                        eng = (
                            nc.gpsimd if (pos + i) in GP_POS else nc.vector
                        )
                        eng.tensor_tensor(
                            w3, e3, vviews[i], mybir.AluOpType.mult
                        )

                        st = pos + i == 0
                        sp = pos + i == K * K - 1
                        for acc, src_t in ((den, eflat), (num, w_t[:])):
                            for h in range(2):
                                base = h * HALF
                                for lo, n in ((0, 512), (512, HALF - 512)):
                                    nc.tensor.matmul(
                                        acc[h][:, lo : lo + n],
                                        ident[:],
                                        src_t[:, base + lo : base + lo + n],
                                        start=st,
                                        stop=sp,
                                    )
                    pos += npr

                # ---- epilogue ----
                out_sb = epool.tile([128, OP], BF16, tag="osb", name=f"osb{c}")
                for h in range(2):
                    base = h * HALF
                    rden = epool.tile([128, HALF], F32, tag="rden", name=f"rd{c}{h}")
                    nc.vector.reciprocal_approx_fast(rden[:], den[h][:])
                    nc.vector.tensor_tensor(
                        out_sb[:, base : base + HALF],
                        num[h][:],
                        rden[:],
                        mybir.AluOpType.mult,
                    )
                nc.sync.dma_start(out_d[c], out_sb[:])

    nc.compile()
    _dedup_ldweights(nc)
    return nc


def _block_diag_weights(w):
    """w: (G, Cg_out, Cg_in) -> lhsT layout [NCH, 128, 128] where
    lhsT[c, ci, co] = w[g, co%32, ci%32] for matching 32-blocks."""
    out = np.zeros((NCH, 128, 128), np.float32)
    for c in range(NCH):
        for g4 in range(4):
            g = c * 4 + g4
            blk = w[g]  # (Cg_out, Cg_in)
            out[c, g4 * 32 : (g4 + 1) * 32, g4 * 32 : (g4 + 1) * 32] = blk.T
    return out


_NC_CACHE = {}


def _make_in_maps(inputs):
    x = np.asarray(inputs["x"], np.float32)
    wq = np.asarray(inputs["wq"], np.float32)
    wk = np.asarray(inputs["wk"], np.float32)
    wv = np.asarray(inputs["wv"], np.float32)
    rel_emb = np.asarray(inputs["rel_emb"], np.float32)
    q_emb = np.asarray(inputs["q_emb"], np.float32)

    bf = ml_dtypes.bfloat16
    wqb = _block_diag_weights(wq).astype(bf)
    wkb = _block_diag_weights(wk).astype(bf)
    wvb = _block_diag_weights(wv).astype(bf)
    relb = np.ascontiguousarray(
        rel_emb.reshape(G, Cg, K * K).reshape(NCH, 128, K * K)
    )
    qeb = np.ascontiguousarray(q_emb.reshape(NCH, 128, 1))
    idn = np.eye(128, dtype=bf)

    xp = np.pad(x, ((0, 0), (P, P), (P, P), (0, 0)))  # (B, 60, 60, C)

    in_maps = []
    for core in range(NCORES):
        b, half = divmod(core, 2)
        sh = xp[b, HS * half : HS * half + MR]         # (32, 60, C)
        xt = np.ascontiguousarray(sh.reshape(SP, C).T).reshape(NCH, 128, SP)
        in_maps.append(
            {
                "xt": xt.astype(bf),
                "wqb": wqb,
                "wkb": wkb,
                "wvb": wvb,
                "relb": relb,
                "qeb": qeb,
                "idn": idn,
            }
        )
    return in_maps


def kernel(**inputs):
    in_maps = _make_in_maps(inputs)

    if "nc" not in _NC_CACHE:
        _NC_CACHE["nc"] = build_nc()
    nc = _NC_CACHE["nc"]

    res = run_bass_kernel_spmd(nc, in_maps, core_ids=list(range(NCORES)))

    out = np.empty((B, H, W, C), np.float32)
    for core in range(NCORES):
        b, half = divmod(core, 2)
        o = np.asarray(res.results[core]["out"]).astype(np.float32)
        o = o.reshape(C, HS, W)
        out[b, HS * half : HS * half + HS] = o.transpose(1, 2, 0)
    return out


# revision 15
# speedup vs baseline: 2.0911x; 1.0237x over previous
"""Trainium2 Bass kernel for AttentionConvFull (local 5x5 window attention
with per-channel softmax, grouped 1x1 conv projections).

Sharding: 8 cores = batch(4) x H-halves(2). Each core gets a 32-row halo'd,
zero-padded slice of x, pre-transposed on host to channel-major [256, 32*60].
No collectives needed.

V2 dataflow per core (2 channel-chunks of 128 partitions each):
  PE    : block-diag 128x128 bf16 matmuls for q/k/v projections; per window
          offset j, identity-matmul PSUM accumulation of den += e_j and
          num += (e_j * v_j).
  DVE   : fused scalar_tensor_tensor t_j = (k_j + rel_j) * q (one pass,
          replaces the separate rel-add); w_j = e_j * v_j tensor_tensor;
          PSUM->SBUF projection copies.
  ACT   : exp over j-PAIRS (batched to amortize per-instr overhead); q
          projection copy with fused q_emb bias.
  GPSIMD: takes a subset of the w_j multiplies to relieve DVE.
  DMA   : 1-elem-shifted copies of k/v maps so odd window columns keep 4B
          alignment (DVE 2x mode); bf16 output (host upcasts to f32).
"""

import numpy as np
import ml_dtypes

import concourse.bass as bass
import concourse.tile as tile
from concourse import bacc, mybir
from concourse.bass_utils import run_bass_kernel_spmd

F32 = mybir.dt.float32
BF16 = mybir.dt.bfloat16

K = 5
G = 8
B, H, W, C = 4, 56, 56, 256
Cg = C // G            # 32
P = K // 2             # 2
HS = H // 2            # 28 output rows per shard
MR = HS + 2 * P        # 32 map rows
MC = W + 2 * P         # 60 map cols
SP = MR * MC           # 1920 map spatial
OP = HS * W            # 1568 output spatial per shard
NCH = 2                # channel chunks of 128 partitions
NCORES = 8
HALF = OP // 2         # 784: PSUM accumulate tile half-size

# NOTE: GPSIMD is useless for offload here: its tensor_scalar is
# pathological (~24us/pass) and its tensor_tensor steals the SBUF port
# shared with DVE, slowing concurrent DVE TTs ~4x (measured).
# which j-iterations (0..24) run their kr-add on ACT (Identity +
# per-partition bias) instead of DVE tensor_scalar, to balance engines
ACT_KR = {0, 2, 4, 7, 9, 11, 13, 15, 18, 20, 22}
# j's per exp batch (one ACT instruction)
EXPB = 4


def _dedup_ldweights(nc):
    """Remove redundant PE weight reloads: consecutive InstLdweights that
    load the same stationary operand with no sync info."""
    removed = 0
    for blk in nc.main_func.blocks:
        last_sig = None
        keep = []
        for inst in blk.instructions:
            if isinstance(inst, mybir.InstLdweights):
                sig = " ".join(a.concise() for a in inst.ins)
                si = inst.sync_info
                clean = si is None or (
                    len(si.on_wait) == 0 and len(si.on_update) == 0
                )
                if sig == last_sig and clean:
                    removed += 1
                    continue
                last_sig = sig
            elif isinstance(inst, mybir.InstMatmult):
                if len(inst.ins) > 1:
                    wsig = inst.ins[1].concise()
                    if wsig != last_sig:
                        last_sig = wsig
            keep.append(inst)
        blk.instructions[:] = keep
    return removed


def build_nc():
    nc = bacc.Bacc(
        "TRN2", target_bir_lowering=False, debug=False, num_devices=NCORES
    )

    xt_d = nc.dram_tensor("xt", [NCH, 128, SP], BF16, kind="ExternalInput").ap()
    wq_d = nc.dram_tensor("wqb", [NCH, 128, 128], BF16, kind="ExternalInput").ap()
    wk_d = nc.dram_tensor("wkb", [NCH, 128, 128], BF16, kind="ExternalInput").ap()
    wv_d = nc.dram_tensor("wvb", [NCH, 128, 128], BF16, kind="ExternalInput").ap()
    rel_d = nc.dram_tensor("relb", [NCH, 128, K * K], F32, kind="ExternalInput").ap()
    qe_d = nc.dram_tensor("qeb", [NCH, 128, 1], F32, kind="ExternalInput").ap()
    id_d = nc.dram_tensor("idn", [128, 128], BF16, kind="ExternalInput").ap()
    out_d = nc.dram_tensor("out", [NCH, 128, OP], BF16, kind="ExternalOutput").ap()

    # dj-major j order: odd-shifted maps (needed from dj=1) arrive via DMA
    # while dj=0 iterations run
    JLIST = [(di, dj) for dj in range(K) for di in range(K)]

    with tile.TileContext(nc) as tc:
        with (
            tc.tile_pool(name="consts", bufs=1) as consts,
            tc.tile_pool(name="weights", bufs=2) as wpool,
            tc.tile_pool(name="xin", bufs=2) as xpool,
            tc.tile_pool(name="maps", bufs=2) as mpool,
            tc.tile_pool(name="jwork", bufs=4) as jpool,
            tc.tile_pool(name="qwork", bufs=2) as qpool,
            tc.tile_pool(name="epi", bufs=2) as epool,
            tc.tile_pool(name="acc", bufs=4, space=bass.MemorySpace.PSUM) as psum,
        ):
            ident = consts.tile([128, 128], BF16, tag="ident")
            nc.sync.dma_start(ident[:], id_d)

            kmaps, komaps, vmaps, vomaps, qflats, rels = [], [], [], [], [], []

            for c in range(NCH):
                x_sb = xpool.tile([128, SP], BF16, tag="x")
                # split the input DMA so the first projection segment can
                # start as soon as its half lands
                hsp = SP // 2
                nc.sync.dma_start(x_sb[:, :hsp], xt_d[c][:, :hsp])
                nc.sync.dma_start(x_sb[:, hsp:], xt_d[c][:, hsp:])

                wts = {}
                for nm, d in (("wq", wq_d), ("wk", wk_d), ("wv", wv_d)):
                    t = wpool.tile([128, 128], BF16, tag=nm, name=f"{nm}{c}")
                    nc.sync.dma_start(t[:], d[c])
                    wts[nm] = t
                rel_sb = wpool.tile([128, K * K], F32, tag="rel", name=f"rel{c}")
                nc.sync.dma_start(rel_sb[:], rel_d[c])
                qe_sb = wpool.tile([128, 1], F32, tag="qe", name=f"qe{c}")
                nc.sync.dma_start(qe_sb[:], qe_d[c])
                rels.append(rel_sb)

                k_bf = mpool.tile([128, SP], BF16, tag="k", name=f"k{c}")
                v_bf = mpool.tile([128, SP], BF16, tag="v", name=f"v{c}")
                qf = mpool.tile([128, OP], BF16, tag="qf", name=f"qf{c}")

                # projections: weight-major to minimize ldweights; 2 psum
                # tiles of 960 cols (16 map rows) each
                NS = 2
                SL = SP // NS  # 960
                for nm in ("wk", "wv", "wq"):
                    for s in range(NS):
                        lo = s * SL
                        rhs = x_sb[:, lo : lo + SL]
                        ps = psum.tile(
                            [128, SL], F32, tag="acc", name=f"pp{c}{s}{nm}"
                        )
                        for mlo, mn in ((0, 512), (512, SL - 512)):
                            nc.tensor.matmul(
                                ps[:, mlo : mlo + mn],
                                wts[nm][:],
                                rhs[:, mlo : mlo + mn],
                                start=True,
                                stop=True,
                            )
                        if nm == "wq":
                            # write the interior (h in [2,30), w in [2,58))
                            # of this 16-row band directly into flat q,
                            # fusing the q_emb per-partition bias (ACT)
                            r0 = max(P, 16 * s)
                            r1 = min(MR - P, 16 * (s + 1))
                            src = ps[:].rearrange("p (h w) -> p h w", h=16)[
                                :, r0 - 16 * s : r1 - 16 * s, P : P + W
                            ]
                            dst = qf[:].rearrange("p (h w) -> p h w", h=HS)[
                                :, r0 - P : r1 - P, :
                            ]
                            nc.scalar.activation(
                                dst,
                                src,
                                mybir.ActivationFunctionType.Identity,
                                bias=qe_sb[:],
                            )
                        elif nm == "wk":
                            nc.scalar.copy(k_bf[:, lo : lo + SL], ps[:])
                        else:
                            nc.scalar.copy(v_bf[:, lo : lo + SL], ps[:])

                # 1-elem-shifted copies (contiguous SBUF->SBUF DMA):
                # x_od[i] = x[i+1], so odd-dj window reads stay 4B-aligned
                k_od = mpool.tile([128, SP], BF16, tag="ko", name=f"ko{c}")
                v_od = mpool.tile([128, SP], BF16, tag="vo", name=f"vo{c}")
                nc.sync.dma_start(k_od[:, : SP - 1], k_bf[:, 1:])
                nc.sync.dma_start(v_od[:, : SP - 1], v_bf[:, 1:])

                kmaps.append(k_bf); komaps.append(k_od)
                vmaps.append(v_bf); vomaps.append(v_od)
                qflats.append(qf)

            # ---- j-loop per chunk ----
            for c in range(NCH):
                rel_sb, qf = rels[c], qflats[c]

                den = [
                    psum.tile([128, HALF], F32, tag="acc", name=f"den{c}{h}")
                    for h in range(2)
                ]
                num = [
                    psum.tile([128, HALF], F32, tag="acc", name=f"num{c}{h}")
                    for h in range(2)
                ]

                # batches of EXPB j positions share one exp instruction;
                # software-pipelined: batch n's kr/t is emitted BEFORE batch
                # n-1's exp so the ACT FIFO never stalls the DVE chain
                batches = [JLIST[i : i + EXPB] for i in range(0, len(JLIST), EXPB)]

                def emit_head(pr, pos):
                    npr = len(pr)
                    t2 = qpool.tile(
                        [128, npr * OP], BF16, tag="t2", name=f"t{c}{pos}"
                    )
                    e2 = qpool.tile(
                        [128, npr * OP], BF16, tag="e2", name=f"e{c}{pos}"
                    )
                    vviews = []
                    # kr = k_j + rel_j (DVE TS 4x / ACT bias), t = kr*q (TT 2x)
                    for i, (di, dj) in enumerate(pr):
                        j = di * K + dj
                        if dj % 2 == 0:
                            ksrc, vsrc, dje = kmaps[c], vmaps[c], dj
                        else:
                            ksrc, vsrc, dje = komaps[c], vomaps[c], dj - 1
                        k3 = ksrc[:].rearrange("p (h w) -> p h w", h=MR)
                        v3 = vsrc[:].rearrange("p (h w) -> p h w", h=MR)
                        kv = k3[:, di : di + HS, dje : dje + W]
                        vviews.append(v3[:, di : di + HS, dje : dje + W])
                        kr_t = jpool.tile(
                            [128, OP], BF16, tag="kr", name=f"kr{c}{pos + i}"
                        )
                        kr3 = kr_t[:].rearrange("p (h w) -> p h w", h=HS)
                        if (pos + i) in ACT_KR:
                            nc.scalar.activation(
                                kr3,
                                kv,
                                mybir.ActivationFunctionType.Identity,
                                bias=rel_sb[:, j : j + 1],
                            )
                        else:
                            nc.vector.tensor_scalar(
                                kr3,
                                kv,
                                rel_sb[:, j : j + 1],
                                None,
                                mybir.AluOpType.add,
                            )
                        nc.vector.tensor_tensor(
                            t2[:, i * OP : (i + 1) * OP],
                            kr_t[:],
                            qf[:],
                            mybir.AluOpType.mult,
                        )
                    return (pr, pos, t2, e2, vviews)

                def emit_tail(staged):
                    pr, pos, t2, e2, vviews = staged
                    nc.scalar.activation(
                        e2[:], t2[:], mybir.ActivationFunctionType.Exp
                    )
                    for i, (di, dj) in enumerate(pr):
                        eflat = e2[:, i * OP : (i + 1) * OP]
                        e3 = eflat.rearrange("p (h w) -> p h w", h=HS)
                        w_t = jpool.tile(
                            [128, OP], BF16, tag="w", name=f"w{c}{pos + i}"
                        )
                        w3 = w_t[:].rearrange("p (h w) -> p h w", h=HS)
                        nc.vector.tensor_tensor(
                            w3, e3, vviews[i], mybir.AluOpType.mult
                        )
                        st = pos + i == 0
                        sp = pos + i == K * K - 1
                        for acc, src_t in ((den, eflat), (num, w_t[:])):
                            for h in range(2):
                                base = h * HALF
                                for lo, n in ((0, 512), (512, HALF - 512)):
                                    nc.tensor.matmul(
                                        acc[h][:, lo : lo + n],
                                        ident[:],
                                        src_t[:, base + lo : base + lo + n],
                                        start=st,
                                        stop=sp,
                                    )

                staged = None
                pos = 0
                for pr in batches:
                    head = emit_head(pr, pos)
                    pos += len(pr)
                    if staged is not None:
                        emit_tail(staged)
                    staged = head
                emit_tail(staged)

                # ---- epilogue ----
                out_sb = epool.tile([128, OP], BF16, tag="osb", name=f"osb{c}")
                for h in range(2):
                    base = h * HALF
                    rden = epool.tile([128, HALF], F32, tag="rden", name=f"rd{c}{h}")
                    nc.vector.reciprocal_approx_fast(rden[:], den[h][:])
                    nc.vector.tensor_tensor(
                        out_sb[:, base : base + HALF],
                        num[h][:],
                        rden[:],
                        mybir.AluOpType.mult,
                    )
                nc.sync.dma_start(out_d[c], out_sb[:])

    nc.compile()
    _dedup_ldweights(nc)
    return nc


def _block_diag_weights(w):
    """w: (G, Cg_out, Cg_in) -> lhsT layout [NCH, 128, 128] where
    lhsT[c, ci, co] = w[g, co%32, ci%32] for matching 32-blocks."""
    out = np.zeros((NCH, 128, 128), np.float32)
    for c in range(NCH):
        for g4 in range(4):
            g = c * 4 + g4
            blk = w[g]  # (Cg_out, Cg_in)
            out[c, g4 * 32 : (g4 + 1) * 32, g4 * 32 : (g4 + 1) * 32] = blk.T
    return out


_NC_CACHE = {}


def _make_in_maps(inputs):
    x = np.asarray(inputs["x"], np.float32)
    wq = np.asarray(inputs["wq"], np.float32)
    wk = np.asarray(inputs["wk"], np.float32)
    wv = np.asarray(inputs["wv"], np.float32)
    rel_emb = np.asarray(inputs["rel_emb"], np.float32)
    q_emb = np.asarray(inputs["q_emb"], np.float32)

    bf = ml_dtypes.bfloat16
    wqb = _block_diag_weights(wq).astype(bf)
    wkb = _block_diag_weights(wk).astype(bf)
    wvb = _block_diag_weights(wv).astype(bf)
    relb = np.ascontiguousarray(
        rel_emb.reshape(G, Cg, K * K).reshape(NCH, 128, K * K)
    )
    qeb = np.ascontiguousarray(q_emb.reshape(NCH, 128, 1))
    idn = np.eye(128, dtype=bf)

    xp = np.pad(x, ((0, 0), (P, P), (P, P), (0, 0)))  # (B, 60, 60, C)

    in_maps = []
    for core in range(NCORES):
        b, half = divmod(core, 2)
        sh = xp[b, HS * half : HS * half + MR]         # (32, 60, C)
        xt = np.ascontiguousarray(sh.reshape(SP, C).T).reshape(NCH, 128, SP)
        in_maps.append(
            {
                "xt": xt.astype(bf),
                "wqb": wqb,
                "wkb": wkb,
                "wvb": wvb,
                "relb": relb,
                "qeb": qeb,
                "idn": idn,
            }
        )
    return in_maps


def kernel(**inputs):
    in_maps = _make_in_maps(inputs)

    if "nc" not in _NC_CACHE:
        _NC_CACHE["nc"] = build_nc()
    nc = _NC_CACHE["nc"]

    res = run_bass_kernel_spmd(nc, in_maps, core_ids=list(range(NCORES)))

    out = np.empty((B, H, W, C), np.float32)
    for core in range(NCORES):
        b, half = divmod(core, 2)
        o = np.asarray(res.results[core]["out"]).astype(np.float32)
        o = o.reshape(C, HS, W)
        out[b, HS * half : HS * half + HS] = o.transpose(1, 2, 0)
    return out


# revision 17
# speedup vs baseline: 2.5365x; 1.2130x over previous
"""Trainium2 Bass kernel for AttentionConvFull (local 5x5 window attention
with per-channel softmax, grouped 1x1 conv projections).

Sharding: 8 cores = batch(4) x H-halves(2). Each core gets a 32-row halo'd,
zero-padded slice of x, pre-transposed on host to channel-major [256, 32*60].
No collectives needed.

V6 dataflow per core (2 channel-chunks of 128 partitions each):
  - j-loop in dj-major order, batches of 5 j's sharing one dj (one source
    map, one shift parity).
  - kr_j = k_j + rel_j: DVE tensor_scalar (4x) / ACT Identity+bias, split
    to balance engines; written into a contiguous quint tile.
  - t = kr * q: ONE tensor_tensor per batch (q broadcast over the 5-j dim
    with a stride-0 AP), 2x mode.
  - e = exp(t): ONE ACT pass per batch (amortizes the ~300ns ACT overhead).
  - w_j = e_j * v_j: per-j DVE TT (2x).
  - den += e_j, num += w_j: identity-matmul PSUM accumulation on PE.
  - Flat software pipeline across (chunk, batch): batch n's kr/t emitted
    before batch n-1's exp/w/matmuls, including across the chunk boundary,
    so no engine FIFO ever serializes the dependency chain.
  - GPSIMD unused: its tensor_scalar is ~24us/pass and its tensor_tensor
    steals the DVE-shared SBUF port (measured 4x DVE TT slowdown).
  - bf16 output, host upcasts.
"""

import numpy as np
import ml_dtypes

import concourse.bass as bass
import concourse.tile as tile
from concourse import bacc, mybir
from concourse.bass_utils import run_bass_kernel_spmd

F32 = mybir.dt.float32
BF16 = mybir.dt.bfloat16

K = 5
G = 8
B, H, W, C = 4, 56, 56, 256
Cg = C // G            # 32
P = K // 2             # 2
HS = H // 2            # 28 output rows per shard
MR = HS + 2 * P        # 32 map rows
MC = W + 2 * P         # 60 map cols
SP = MR * MC           # 1920 map spatial
OP = HS * W            # 1568 output spatial per shard
NCH = 2                # channel chunks of 128 partitions
NCORES = 8
HALF = OP // 2         # 784: PSUM accumulate tile half-size

# positions (0..24 within a chunk, dj-major) whose kr-add runs on ACT
# (Identity + per-partition bias) instead of DVE tensor_scalar
ACT_KR = {0, 2, 4, 6, 8, 11, 13, 15, 17, 19, 21, 22, 23, 24}
QUINT_T = True         # one t-TT per batch with q broadcast over the j dim


def _dedup_ldweights(nc):
    """Remove redundant PE weight reloads: consecutive InstLdweights that
    load the same stationary operand with no sync info."""
    removed = 0
    for blk in nc.main_func.blocks:
        last_sig = None
        keep = []
        for inst in blk.instructions:
            if isinstance(inst, mybir.InstLdweights):
                sig = " ".join(a.concise() for a in inst.ins)
                si = inst.sync_info
                clean = si is None or (
                    len(si.on_wait) == 0 and len(si.on_update) == 0
                )
                if sig == last_sig and clean:
                    removed += 1
                    continue
                last_sig = sig
            elif isinstance(inst, mybir.InstMatmult):
                if len(inst.ins) > 1:
                    wsig = inst.ins[1].concise()
                    if wsig != last_sig:
                        last_sig = wsig
            keep.append(inst)
        blk.instructions[:] = keep
    return removed


def build_nc():
    nc = bacc.Bacc(
        "TRN2", target_bir_lowering=False, debug=False, num_devices=NCORES
    )

    xt_d = nc.dram_tensor("xt", [NCH, 128, SP], BF16, kind="ExternalInput").ap()
    wq_d = nc.dram_tensor("wqb", [NCH, 128, 128], BF16, kind="ExternalInput").ap()
    wk_d = nc.dram_tensor("wkb", [NCH, 128, 128], BF16, kind="ExternalInput").ap()
    wv_d = nc.dram_tensor("wvb", [NCH, 128, 128], BF16, kind="ExternalInput").ap()
    rel_d = nc.dram_tensor("relb", [NCH, 128, K * K], F32, kind="ExternalInput").ap()
    qe_d = nc.dram_tensor("qeb", [NCH, 128, 1], F32, kind="ExternalInput").ap()
    id_d = nc.dram_tensor("idn", [128, 128], BF16, kind="ExternalInput").ap()
    out_d = nc.dram_tensor("out", [NCH, 128, OP], BF16, kind="ExternalOutput").ap()

    # dj-major: batch b = all di for one dj; odd-shifted maps (dj 1,3)
    # arrive via DMA while the dj=0 batch runs
    JLIST = [(di, dj) for dj in range(K) for di in range(K)]

    with tile.TileContext(nc) as tc:
        with (
            tc.tile_pool(name="consts", bufs=1) as consts,
            tc.tile_pool(name="weights", bufs=2) as wpool,
            tc.tile_pool(name="xin", bufs=2) as xpool,
            tc.tile_pool(name="maps", bufs=2) as mpool,
            tc.tile_pool(name="jwork", bufs=4) as jpool,
            tc.tile_pool(name="qwork", bufs=2) as qpool,
            tc.tile_pool(name="krw", bufs=2) as krpool,
            tc.tile_pool(name="epi", bufs=2) as epool,
            tc.tile_pool(name="acc", bufs=4, space=bass.MemorySpace.PSUM) as psum,
        ):
            ident = consts.tile([128, 128], BF16, tag="ident")
            nc.sync.dma_start(ident[:], id_d)

            kmaps, komaps, vmaps, vomaps, qflats, rels = [], [], [], [], [], []

            for c in range(NCH):
                x_sb = xpool.tile([128, SP], BF16, tag="x")
                hsp = SP // 2
                nc.sync.dma_start(x_sb[:, :hsp], xt_d[c][:, :hsp])
                nc.sync.dma_start(x_sb[:, hsp:], xt_d[c][:, hsp:])

                wts = {}
                for nm, d in (("wq", wq_d), ("wk", wk_d), ("wv", wv_d)):
                    t = wpool.tile([128, 128], BF16, tag=nm, name=f"{nm}{c}")
                    nc.sync.dma_start(t[:], d[c])
                    wts[nm] = t
                rel_sb = wpool.tile([128, K * K], F32, tag="rel", name=f"rel{c}")
                nc.sync.dma_start(rel_sb[:], rel_d[c])
                qe_sb = wpool.tile([128, 1], F32, tag="qe", name=f"qe{c}")
                nc.sync.dma_start(qe_sb[:], qe_d[c])
                rels.append(rel_sb)

                k_bf = mpool.tile([128, SP], BF16, tag="k", name=f"k{c}")
                v_bf = mpool.tile([128, SP], BF16, tag="v", name=f"v{c}")
                qf = mpool.tile([128, OP], BF16, tag="qf", name=f"qf{c}")

                # projections: k first then q (so the j-loop head can start),
                # then v (only needed by the first w-mult); weight-major to
                # minimize ldweights; 2 psum tiles of 960 cols each
                NS = 2
                SL = SP // NS  # 960
                for nm in ("wk", "wq", "wv"):
                    for s in range(NS):
                        lo = s * SL
                        rhs = x_sb[:, lo : lo + SL]
                        ps = psum.tile(
                            [128, SL], F32, tag="acc", name=f"pp{c}{s}{nm}"
                        )
                        for mlo, mn in ((0, 512), (512, SL - 512)):
                            nc.tensor.matmul(
                                ps[:, mlo : mlo + mn],
                                wts[nm][:],
                                rhs[:, mlo : mlo + mn],
                                start=True,
                                stop=True,
                            )
                        if nm == "wq":
                            # write the interior (h in [2,30), w in [2,58))
                            # of this 16-row band directly into flat q,
                            # fusing the q_emb per-partition bias (ACT)
                            r0 = max(P, 16 * s)
                            r1 = min(MR - P, 16 * (s + 1))
                            src = ps[:].rearrange("p (h w) -> p h w", h=16)[
                                :, r0 - 16 * s : r1 - 16 * s, P : P + W
                            ]
                            dst = qf[:].rearrange("p (h w) -> p h w", h=HS)[
                                :, r0 - P : r1 - P, :
                            ]
                            nc.scalar.activation(
                                dst,
                                src,
                                mybir.ActivationFunctionType.Identity,
                                bias=qe_sb[:],
                            )
                        elif nm == "wk":
                            nc.scalar.copy(k_bf[:, lo : lo + SL], ps[:])
                        else:
                            nc.scalar.copy(v_bf[:, lo : lo + SL], ps[:])

                # 1-elem-shifted copies (contiguous SBUF->SBUF DMA):
                # x_od[i] = x[i+1], so odd-dj window reads stay 4B-aligned
                k_od = mpool.tile([128, SP], BF16, tag="ko", name=f"ko{c}")
                v_od = mpool.tile([128, SP], BF16, tag="vo", name=f"vo{c}")
                nc.sync.dma_start(k_od[:, : SP - 1], k_bf[:, 1:])
                nc.sync.dma_start(v_od[:, : SP - 1], v_bf[:, 1:])

                kmaps.append(k_bf); komaps.append(k_od)
                vmaps.append(v_bf); vomaps.append(v_od)
                qflats.append(qf)

            # ---- flat software-pipelined j-loop over (chunk, dj-batch) ----
            dens, nums = {}, {}

            def emit_head(c, pr, pos):
                rel_sb, qf = rels[c], qflats[c]
                npr = len(pr)
                t2 = qpool.tile([128, npr * OP], BF16, tag="t2", name=f"t{c}{pos}")
                e2 = qpool.tile([128, npr * OP], BF16, tag="e2", name=f"e{c}{pos}")
                kr_t = krpool.tile(
                    [128, npr * OP], BF16, tag="kr", name=f"kr{c}{pos}"
                )
                vviews = []
                for i, (di, dj) in enumerate(pr):
                    j = di * K + dj
                    if dj % 2 == 0:
                        ksrc, vsrc, dje = kmaps[c], vmaps[c], dj
                    else:
                        ksrc, vsrc, dje = komaps[c], vomaps[c], dj - 1
                    k3 = ksrc[:].rearrange("p (h w) -> p h w", h=MR)
                    v3 = vsrc[:].rearrange("p (h w) -> p h w", h=MR)
                    kv = k3[:, di : di + HS, dje : dje + W]
                    vviews.append(v3[:, di : di + HS, dje : dje + W])
                    kr3 = kr_t[:, i * OP : (i + 1) * OP].rearrange(
                        "p (h w) -> p h w", h=HS
                    )
                    if (pos + i) % 25 in ACT_KR:
                        nc.scalar.activation(
                            kr3,
                            kv,
                            mybir.ActivationFunctionType.Identity,
                            bias=rel_sb[:, j : j + 1],
                        )
                    else:
                        nc.vector.tensor_scalar(
                            kr3,
                            kv,
                            rel_sb[:, j : j + 1],
                            None,
                            mybir.AluOpType.add,
                        )
                if QUINT_T:
                    qb = qf[:].unsqueeze(1).broadcast_to([128, npr, OP])
                    nc.vector.tensor_tensor(
                        t2[:].rearrange("p (s f) -> p s f", s=npr),
                        kr_t[:].rearrange("p (s f) -> p s f", s=npr),
                        qb,
                        mybir.AluOpType.mult,
                    )
                else:
                    for i in range(npr):
                        nc.vector.tensor_tensor(
                            t2[:, i * OP : (i + 1) * OP],
                            kr_t[:, i * OP : (i + 1) * OP],
                            qf[:],
                            mybir.AluOpType.mult,
                        )
                return (c, pr, pos, t2, e2, vviews)

            def emit_tail(staged):
                c, pr, pos, t2, e2, vviews = staged
                den, num = dens[c], nums[c]
                nc.scalar.activation(
                    e2[:], t2[:], mybir.ActivationFunctionType.Exp
                )
                for i, (di, dj) in enumerate(pr):
                    eflat = e2[:, i * OP : (i + 1) * OP]
                    e3 = eflat.rearrange("p (h w) -> p h w", h=HS)
                    w_t = jpool.tile(
                        [128, OP], BF16, tag="w", name=f"w{c}{pos + i}"
                    )
                    w3 = w_t[:].rearrange("p (h w) -> p h w", h=HS)
                    nc.vector.tensor_tensor(
                        w3, e3, vviews[i], mybir.AluOpType.mult
                    )
                    st = pos + i == 0
                    sp = pos + i == K * K - 1
                    for acc, src_t in ((den, eflat), (num, w_t[:])):
                        for h in range(2):
                            base = h * HALF
                            for lo, n in ((0, 512), (512, HALF - 512)):
                                nc.tensor.matmul(
                                    acc[h][:, lo : lo + n],
                                    ident[:],
                                    src_t[:, base + lo : base + lo + n],
                                    start=st,
                                    stop=sp,
                                )

            def emit_epilogue(c):
                den, num = dens[c], nums[c]
                out_sb = epool.tile([128, OP], BF16, tag="osb", name=f"osb{c}")
                for h in range(2):
                    base = h * HALF
                    rden = epool.tile(
                        [128, HALF], F32, tag="rden", name=f"rd{c}{h}"
                    )
                    nc.vector.reciprocal_approx_fast(rden[:], den[h][:])
                    nc.vector.tensor_tensor(
                        out_sb[:, base : base + HALF],
                        num[h][:],
                        rden[:],
                        mybir.AluOpType.mult,
                    )
                nc.sync.dma_start(out_d[c], out_sb[:])

            worklist = []
            for c in range(NCH):
                for b in range(K):
                    worklist.append((c, JLIST[b * K : (b + 1) * K], b * K))

            staged = None
            for c, pr, pos in worklist:
                if pos == 0:
                    dens[c] = [
                        psum.tile([128, HALF], F32, tag="acc", name=f"den{c}{h}")
                        for h in range(2)
                    ]
                    nums[c] = [
                        psum.tile([128, HALF], F32, tag="acc", name=f"num{c}{h}")
                        for h in range(2)
                    ]
                head = emit_head(c, pr, pos)
                if staged is not None:
                    emit_tail(staged)
                    if staged[2] == K * K - K and staged[0] != c:
                        emit_epilogue(staged[0])
                staged = head
            emit_tail(staged)
            emit_epilogue(staged[0])

    nc.compile()
    _dedup_ldweights(nc)
    return nc


def _block_diag_weights(w):
    """w: (G, Cg_out, Cg_in) -> lhsT layout [NCH, 128, 128] where
    lhsT[c, ci, co] = w[g, co%32, ci%32] for matching 32-blocks."""
    out = np.zeros((NCH, 128, 128), np.float32)
    for c in range(NCH):
        for g4 in range(4):
            g = c * 4 + g4
            blk = w[g]  # (Cg_out, Cg_in)
            out[c, g4 * 32 : (g4 + 1) * 32, g4 * 32 : (g4 + 1) * 32] = blk.T
    return out


_NC_CACHE = {}


def _make_in_maps(inputs):
    x = np.asarray(inputs["x"], np.float32)
    wq = np.asarray(inputs["wq"], np.float32)
    wk = np.asarray(inputs["wk"], np.float32)
    wv = np.asarray(inputs["wv"], np.float32)
    rel_emb = np.asarray(inputs["rel_emb"], np.float32)
    q_emb = np.asarray(inputs["q_emb"], np.float32)

    bf = ml_dtypes.bfloat16
    wqb = _block_diag_weights(wq).astype(bf)
    wkb = _block_diag_weights(wk).astype(bf)
    wvb = _block_diag_weights(wv).astype(bf)
    relb = np.ascontiguousarray(
        rel_emb.reshape(G, Cg, K * K).reshape(NCH, 128, K * K)
    )
    qeb = np.ascontiguousarray(q_emb.reshape(NCH, 128, 1))
    idn = np.eye(128, dtype=bf)

    xp = np.pad(x, ((0, 0), (P, P), (P, P), (0, 0)))  # (B, 60, 60, C)

    in_maps = []
    for core in range(NCORES):
        b, half = divmod(core, 2)
        sh = xp[b, HS * half : HS * half + MR]         # (32, 60, C)
        xt = np.ascontiguousarray(sh.reshape(SP, C).T).reshape(NCH, 128, SP)
        in_maps.append(
            {
                "xt": xt.astype(bf),
                "wqb": wqb,
                "wkb": wkb,
                "wvb": wvb,
                "relb": relb,
                "qeb": qeb,
                "idn": idn,
            }
        )
    return in_maps


def kernel(**inputs):
    in_maps = _make_in_maps(inputs)

    if "nc" not in _NC_CACHE:
        _NC_CACHE["nc"] = build_nc()
    nc = _NC_CACHE["nc"]

    res = run_bass_kernel_spmd(nc, in_maps, core_ids=list(range(NCORES)))

    out = np.empty((B, H, W, C), np.float32)
    for core in range(NCORES):
        b, half = divmod(core, 2)
        o = np.asarray(res.results[core]["out"]).astype(np.float32)
        o = o.reshape(C, HS, W)
        out[b, HS * half : HS * half + HS] = o.transpose(1, 2, 0)
    return out
